# revision 22
# baseline (speedup 1.0000x reference)
"""Trainium2 Bass kernel for nn_MultiHeadAttention_36009005810143.

Data-parallel over batch B=8 across 8 NeuronCores; projection weights
replicated.  Per core: x [1024,640] -> MHA (10 heads, d=64, strict
causal additive -10000 mask) -> out [1024,640] * mask.

v3 design:
 - x transposed on the host; projections read x^T directly.
 - S^T = K_h^T Q_h causally trimmed (only q >= 128*kb per k-block),
   packed into bank-aligned slots of a [128, 2048] psum tile whose
   halves hold the two heads of a pair.  Head pairs live at partition
   offsets 0/64 of QT/KT, so the K=64-contraction S matmuls row-tile
   both heads concurrently on the PE array.
 - exp runs straight from PSUM in one ACTIVATE per 2048-col tile
   (scale=1/8); the causal band is zeroed AFTER exp on the gpsimd
   engine (exp(s-10000) == 0 exactly).
 - PV keeps the S^T orientation: stationary = [V_h | 1] (one cheap
   65-col LDWEIGHTS per (head, k-block)), moving = exp'd attention.
   Output lands transposed ([U, T]); the softmax denominator is row 64
   of each PV psum tile.  Den rows are DMA-gathered into a [2, T]
   staging tile, rden = mask / den in one DVE divide (folds the query
   mask: mask=0 -> 0), broadcast across partitions by the DMA engine,
   and applied in a single [64, T] multiply per head.  The host
   transposes the returned [U, T] array (layout only).
 - reference quirk: for q==0 every key gets -10000, so softmax over
   ALL 1024 keys of the raw scores.  S^T columns q=0..7 for k-blocks
   1..7 are 8-wide strips (cols 1..7 zeroed post-exp) accumulated into
   columns 0..7 of the qc=0 PV psum.
"""

import os
import sys
import types

import numpy as np

# The agent image's `antenv` package lacks `axon_hooks`, which
# concourse.bass_utils imports unconditionally when trace=True under
# axon.  Provide it (and register the real NTFF hook when available).
try:
    import antenv

    if not hasattr(antenv, "axon_hooks"):
        _hooks_mod = types.ModuleType("antenv.axon_hooks")
        _hooks_mod._hook = None

        def _set_hook(h):
            _hooks_mod._hook = h

        def _get_hook():
            return _hooks_mod._hook

        _hooks_mod.set_axon_ntff_profile_hook = _set_hook
        _hooks_mod.get_axon_ntff_profile_hook = _get_hook
        sys.modules["antenv.axon_hooks"] = _hooks_mod
        antenv.axon_hooks = _hooks_mod
        try:
            from trn_agent_boot.trn_boot import _ntff_profile_via_ctypes

            _set_hook(_ntff_profile_via_ctypes("/opt/axon/libaxon_pjrt.so"))
        except Exception:
            pass
except Exception:
    pass

import concourse.bass as bass
import concourse.mybir as mybir
import concourse.tile as tile
from concourse import bacc
from concourse.bass_utils import run_bass_kernel_spmd

F32 = mybir.dt.float32
F16 = mybir.dt.float16
AF = mybir.ActivationFunctionType

B, T, D, U, H, DH = 8, 1024, 640, 640, 10, 64
NTB = T // 128   # 8 t/q/k 128-blocks
NDB = D // 128   # 5 contraction blocks
NUB = U // 128   # 5 feature blocks
VCW = 320        # V projection chunk (5 heads)
PAIRS = H // 2   # 5 row-tiled head pairs
HD1 = DH + 1     # 65: head cols in Vg / PV psum partitions

# Per-head S^T slot layout: 5 logical tiles of [128, 1024] packed with
# causally-needed (kb, q-range) chunks; every matmul stays inside one
# 2KB psum bank.  Entries: (tile_idx, col_off, kb, q_lo, width)
CHUNKS = [
    (0, 0,   0, 0,   512), (0, 512, 0, 512, 512),
    (1, 0,   1, 128, 512), (1, 512, 1, 640, 384), (1, 896, 7, 896, 128),
    (2, 0,   2, 256, 512), (2, 512, 2, 768, 256), (2, 768, 6, 768, 256),
    (3, 0,   3, 384, 512), (3, 512, 3, 896, 128), (3, 640, 5, 640, 384),
    (4, 0,   4, 512, 512),
]
NS = 5           # S tiles per head
SW = 1024        # per-head S tile width (half of the [128,2048] psum)
STRIP0 = 512     # strip cols in tile 4: kb=1..7 at STRIP0 + 8*(kb-1)

# piece map: kb -> list of (tile, col_off, q_lo, q_hi)
_PIECES = {kb: [] for kb in range(NTB)}
for (ti, co, kb, qlo, w) in CHUNKS:
    _PIECES[kb].append((ti, co, qlo, qlo + w))

# diag regions: (tile, col_off) of the 128-wide diagonal block per kb
DIAG = {}
for (ti, co, kb, qlo, w) in CHUNKS:
    if qlo == 128 * kb:
        DIAG[kb] = (ti, co)


def _ptoff(hi, ti):
    """p_t column base for head-in-pair hi, S tile ti."""
    return 2048 * ti + 1024 * hi


def _pv_segments(qc):
    """PV moving-operand segments for one 512-col q chunk.
    Returns [(kb, ti, col_in_tile, psum_off, width)] with kb ascending;
    kb=0 comes first and spans the full 512 columns."""
    segs = []
    for kb in range(NTB):
        qlo, qhi = max(512 * qc, 128 * kb), 512 * (qc + 1)
        if qlo >= qhi:
            continue
        for (ti, co, plo, phi) in _PIECES[kb]:
            a, b = max(qlo, plo), min(qhi, phi)
            if a < b:
                segs.append((kb, ti, co + (a - plo), a - 512 * qc, b - a))
    return segs


PV_SEGS = {qc: _pv_segments(qc) for qc in range(2)}

_CACHE: dict = {}


def _build_module():
    nc = bacc.Bacc("TRN2", target_bir_lowering=False, debug=False, num_devices=B)

    xT_d = nc.dram_tensor("x", [D, T], F16, kind="ExternalInput").ap()
    m_d = nc.dram_tensor("mask", [1, T], F32, kind="ExternalInput").ap()
    wq_d = nc.dram_tensor("Wq", [D, U], F16, kind="ExternalInput").ap()
    wk_d = nc.dram_tensor("Wk", [D, U], F16, kind="ExternalInput").ap()
    wv_d = nc.dram_tensor("Wv", [D, U], F16, kind="ExternalInput").ap()
    out_d = nc.dram_tensor("out", [U, T], F16, kind="ExternalOutput").ap()

    ts = bass.ts

    with tile.TileContext(nc) as tc:
        from contextlib import ExitStack

        with ExitStack() as ctx:
            consts = ctx.enter_context(tc.tile_pool(name="consts", bufs=1))
            sb = ctx.enter_context(tc.tile_pool(name="sb", bufs=1))
            ptp = ctx.enter_context(tc.tile_pool(name="ptp", bufs=2))
            otp = ctx.enter_context(tc.tile_pool(name="otp", bufs=3))
            ofp = ctx.enter_context(tc.tile_pool(name="ofp", bufs=3))
            rbp = ctx.enter_context(tc.tile_pool(name="rbp", bufs=3))
            dnp = ctx.enter_context(tc.tile_pool(name="dnp", bufs=2))
            pp = ctx.enter_context(tc.tile_pool(name="pp", bufs=2, space="PSUM"))
            spp = ctx.enter_context(tc.tile_pool(name="spp", bufs=1, space="PSUM"))
            pvp = ctx.enter_context(tc.tile_pool(name="pvp", bufs=2, space="PSUM"))

            # ---------------- input DMAs (x^T, W first) ----------------
            xT = [sb.tile([128, T], F16, tag=f"xT{i}", name=f"xT{i}") for i in range(NDB)]
            Wq = [sb.tile([128, U], F16, tag=f"wq{i}", name=f"wq{i}") for i in range(NDB)]
            Wk = [sb.tile([128, U], F16, tag=f"wk{i}", name=f"wk{i}") for i in range(NDB)]
            Wv = [sb.tile([128, U], F16, tag=f"wv{i}", name=f"wv{i}") for i in range(NDB)]
            QT = [sb.tile([128, T], F16, tag=f"QT{i}", name=f"QT{i}") for i in range(NUB)]
            KT = [sb.tile([128, T], F16, tag=f"KT{i}", name=f"KT{i}") for i in range(NUB)]
            Vg = [sb.tile([128, H * HD1], F16, tag=f"Vg{i}", name=f"Vg{i}")
                  for i in range(NTB)]

            # interleave x^T/Wq tiles so the first Q projection chain can
            # stream while later tiles are still in flight; split across
            # the two hardware DMA issue queues (sync + scalar)
            for i in range(NDB):
                nc.scalar.dma_start(xT[i][:], xT_d[ts(i, 128), :])
                nc.scalar.dma_start(Wq[i][:], wq_d[ts(i, 128), :])
            for i in range(NDB):
                nc.sync.dma_start(Wk[i][:], wk_d[ts(i, 128), :])
            for i in range(NDB):
                nc.sync.dma_start(Wv[i][:], wv_d[ts(i, 128), :])

            mrow = consts.tile([1, T], F32, name="mrow")
            nc.sync.dma_start(mrow[:], m_d[:, :])
            # query mask broadcast across the PV psum partitions; row 64
            # stays 1.0 so the denominator is never masked (avoids 0/0).
            maskb = consts.tile([HD1, T], F32, name="maskb")
            nc.gpsimd.partition_broadcast(maskb[:], mrow[:], channels=HD1)
            nc.gpsimd.memset(maskb[DH:HD1, :], 1.0)

            ones_t = consts.tile([128, H], F32, name="ones_t")
            nc.vector.memset(ones_t[:], 1.0)
            ones64 = consts.tile([1, DH], F32, name="ones64")
            nc.vector.memset(ones64[:], 1.0)
            for tb in range(NTB):
                oc = Vg[tb][:].rearrange("p (g c) -> p g c", c=HD1)[:, :, DH:HD1]
                nc.vector.tensor_copy(
                    oc, ones_t[:].rearrange("p (g c) -> p g c", c=1))

            # ---------------- projection helpers -----------------------
            def proj_qk(W, dst, ub, qc, drain_act=False):
                ps = pp.tile([128, 512], F32, tag="pp", name="pp")
                for db in range(NDB):
                    nc.tensor.matmul(
                        ps[:], W[db][:, ts(ub, 128)], xT[db][:, ts(qc, 512)],
                        start=(db == 0), stop=(db == NDB - 1))
                if drain_act:
                    nc.scalar.activation(dst[ub][:, ts(qc, 512)], ps[:], AF.Copy)
                else:
                    nc.vector.tensor_copy(dst[ub][:, ts(qc, 512)], ps[:])

            def proj_v(tb, vc, drain_act=False):
                ps = pp.tile([128, 512], F32, tag="pp", name="pp")
                for db in range(NDB):
                    nc.tensor.matmul(
                        ps[:, 0:VCW], xT[db][:, ts(tb, 128)],
                        Wv[db][:, ts(vc, VCW)],
                        start=(db == 0), stop=(db == NDB - 1))
                dst = Vg[tb][:, vc * 5 * HD1:(vc + 1) * 5 * HD1]
                dst = dst.rearrange("p (g c) -> p g c", c=HD1)[:, :, 0:DH]
                src = ps[:, 0:VCW].rearrange("p (g c) -> p g c", c=DH)
                if drain_act:
                    nc.scalar.activation(dst, src, AF.Copy)
                else:
                    nc.vector.tensor_copy(dst, src)

            # ---------------- attention emission -----------------------
            PB = {}    # pair -> ptAB tile

            def emit_s_tile(pair, ti):
                kt = KT[pair]
                qt = QT[pair]
                sAB = spp.tile([128, 2048], F32, tag="sAB", name="sAB")
                for (t2, co, kb, qlo, w) in CHUNKS:
                    if t2 != ti:
                        continue
                    for hi in range(2):
                        po = DH * hi
                        nc.tensor.matmul(
                            sAB[:, 1024 * hi + co:1024 * hi + co + w],
                            kt[po:po + DH, ts(kb, 128)],
                            qt[po:po + DH, qlo:qlo + w],
                            start=True, stop=True)
                if ti == 4:
                    for kb in range(1, NTB):
                        co = STRIP0 + 8 * (kb - 1)
                        for hi in range(2):
                            po = DH * hi
                            nc.tensor.matmul(
                                sAB[:, 1024 * hi + co:1024 * hi + co + 8],
                                kt[po:po + DH, ts(kb, 128)],
                                qt[po:po + DH, 0:8],
                                start=True, stop=True)
                return sAB

            def emit_exp_tile(pair, ti, sAB):
                pt = PB[pair]
                if ti < 4:
                    nc.scalar.activation(
                        pt[:, 2048 * ti:2048 * (ti + 1)], sAB[:],
                        AF.Exp, scale=0.125)
                else:
                    w = STRIP0 + 56
                    for hi in range(2):
                        nc.scalar.activation(
                            pt[:, _ptoff(hi, 4):_ptoff(hi, 4) + w],
                            sAB[:, 1024 * hi:1024 * hi + w],
                            AF.Exp, scale=0.125)
                # post-exp band zeroing on gpsimd
                for hi in range(2):
                    for kb in range(NTB):
                        t2, co = DIAG[kb]
                        if t2 != ti:
                            continue
                        base = _ptoff(hi, ti) + co
                        if kb == 0:
                            # keep col q=0 (reference quirk); cols 1..127:
                            # keep iff (c+1) > p  <=>  c - p >= 0
                            nc.gpsimd.affine_select(
                                out=pt[:, base + 1:base + 128],
                                in_=pt[:, base + 1:base + 128],
                                compare_op=mybir.AluOpType.is_ge,
                                fill=0.0, base=0,
                                pattern=[[1, 127]], channel_multiplier=-1)
                        else:
                            # keep iff c - p - 1 >= 0 (strict causal)
                            nc.gpsimd.affine_select(
                                out=pt[:, base:base + 128],
                                in_=pt[:, base:base + 128],
                                compare_op=mybir.AluOpType.is_ge,
                                fill=0.0, base=-1,
                                pattern=[[1, 128]], channel_multiplier=-1)
                    if ti == 4:
                        sb0 = _ptoff(hi, 4) + STRIP0
                        # zero cols 1..7 of each 8-wide q0 strip
                        nc.gpsimd.affine_select(
                            out=pt[:, sb0:sb0 + 56].rearrange(
                                "p (g c) -> p g c", c=8),
                            in_=pt[:, sb0:sb0 + 56].rearrange(
                                "p (g c) -> p g c", c=8),
                            compare_op=mybir.AluOpType.is_ge,
                            fill=0.0, base=0,
                            pattern=[[0, 7], [-1, 8]], channel_multiplier=0)

            def emit_pv_chain(pair, hi, qc, oT, sden):
                """PV chain for head 2*pair+hi over q chunk qc; drains into
                oT with the query mask folded in, then DMA-gathers the den
                row into the head's base-0 staging tile."""
                pt = PB[pair]
                h = 2 * pair + hi
                pv = pvp.tile([HD1, 512], F32, tag="pv", name="pv")
                vsl = (h * HD1, (h + 1) * HD1)
                segs = PV_SEGS[qc]
                for j, (kb, ti, c, off, w) in enumerate(segs):
                    nc.tensor.matmul(
                        pv[:, off:off + w],
                        Vg[kb][:, vsl[0]:vsl[1]],
                        pt[:, _ptoff(hi, ti) + c:_ptoff(hi, ti) + c + w],
                        start=(j == 0),
                        stop=(qc == 1 and j == len(segs) - 1))
                if qc == 0:
                    for kb in range(1, NTB):
                        c = _ptoff(hi, 4) + STRIP0 + 8 * (kb - 1)
                        nc.tensor.matmul(
                            pv[:, 0:8],
                            Vg[kb][:, vsl[0]:vsl[1]],
                            pt[:, c:c + 8],
                            start=False, stop=(kb == NTB - 1))
                nc.vector.tensor_tensor(
                    oT[:, ts(qc, 512)], pv[:], maskb[:, ts(qc, 512)],
                    op=mybir.AluOpType.mult)
                nc.sync.dma_start(
                    sden[:, ts(qc, 512)], oT[64:65, ts(qc, 512)])

            HS = {}  # (pair, hi) -> (oT, sden) from the chain unit

            def emit_head_chains(pair, hi):
                oT = otp.tile([HD1, T], F32, tag="oT", name="oT")
                sden = dnp.tile([1, T], F32, tag="sden", name="sden")
                for qc in range(2):
                    emit_pv_chain(pair, hi, qc, oT, sden)
                HS[(pair, hi)] = (oT, sden)

            def emit_head_norm(pair, hi):
                oT, sden = HS.pop((pair, hi))
                # broadcast den across partitions via a rank-1 fp32 matmul
                # (ones[1,64]^T @ den[1,512]) -- runs on the PE, which has
                # slack late in the schedule, instead of clogging gpsimd
                dbc = rbp.tile([DH, T], F32, tag="dbc", name="dbc")
                for qc in range(2):
                    bc = pvp.tile([HD1, 512], F32, tag="pv", name="pv")
                    nc.tensor.matmul(
                        bc[0:DH, :], ones64[:], sden[:, ts(qc, 512)],
                        start=True, stop=True)
                    # den is a sum of exps: positive, finite -> approx ok
                    nc.vector.reciprocal_approx_fast(
                        dbc[:, ts(qc, 512)], bc[0:DH, :])
                oF = ofp.tile([DH, T], F16, tag="oF", name="oF")
                nc.vector.tensor_tensor(
                    oF[:], oT[0:DH, :], dbc[:],
                    op=mybir.AluOpType.mult)
                h = 2 * pair + hi
                nc.sync.dma_start(out_d[h * DH:(h + 1) * DH, :], oF[:])

            # ------------------- emission schedule ---------------------
            proj_qk(Wq, QT, 0, 0, True); proj_qk(Wq, QT, 0, 1, True)
            proj_qk(Wk, KT, 0, 0, True); proj_qk(Wk, KT, 0, 1, True)
            for tb in range(NTB):
                proj_v(tb, 0, drain_act=True)

            # deferred work: (est_pe_us, needed_by_pair, emit_fn)
            work = []
            for ub in range(1, NUB):
                for (W, DST) in ((Wq, QT), (Wk, KT)):
                    for qc in range(2):
                        work.append((1.1, ub, lambda W=W, D=DST, u=ub, q=qc:
                                     proj_qk(W, D, u, q)))
            for tb in range(NTB):
                work.append((0.7, None, lambda t=tb: proj_v(t, 1)))

            def emit_work(budget_us):
                spent = 0.0
                while work and spent < budget_us:
                    cost, _, fn = work.pop(0)
                    fn()
                    spent += cost

            def flush_proj_for(pair):
                keep = []
                for (cost, ub, fn) in work:
                    if ub is not None and ub <= pair:
                        fn()
                    else:
                        keep.append((cost, ub, fn))
                work[:] = keep

            for pair in range(PAIRS):
                PB[pair] = ptp.tile([128, 2048 * NS], F16, tag="ptAB", name="ptAB")
                flush_proj_for(pair)
                for ti in range(NS):
                    sAB = emit_s_tile(pair, ti)
                    emit_exp_tile(pair, ti, sAB)
                    emit_work(2.2)
                # pair's own head A becomes available right after its exps;
                # queue it first so the next gaps pick it up immediately
                work.insert(0, (2.0, None,
                                lambda p=pair: emit_head_chains(p, 0)))
                work.insert(1, (0.3, None,
                                lambda p=pair: emit_head_norm(p, 0)))
                work.append((2.0, None,
                             lambda p=pair: emit_head_chains(p, 1)))
                work.append((0.3, None,
                             lambda p=pair: emit_head_norm(p, 1)))
                emit_work(2.3)

            while work:
                _, _, fn = work.pop(0)
                fn()

    nc.compile()
    return nc


def get_nc():
    if "nc" not in _CACHE:
        _CACHE["nc"] = _build_module()
    return _CACHE["nc"]


def kernel(x, mask, Wq, Wk, Wv):
    x = np.asarray(x, dtype=np.float32).astype(np.float16)
    xT = np.ascontiguousarray(x.transpose(0, 2, 1))  # [B, D, T]
    mask_f = np.ascontiguousarray(
        np.asarray(mask).astype(np.float32).reshape(B, 1, T))
    Wq = np.ascontiguousarray(np.asarray(Wq, dtype=np.float32).astype(np.float16))
    Wk = np.ascontiguousarray(np.asarray(Wk, dtype=np.float32).astype(np.float16))
    Wv = np.ascontiguousarray(np.asarray(Wv, dtype=np.float32).astype(np.float16))

    nc = get_nc()
    in_maps = [
        {"x": xT[b], "mask": mask_f[b], "Wq": Wq, "Wk": Wk, "Wv": Wv}
        for b in range(B)
    ]
    trace = bool(int(os.environ.get("KERNEL_TRACE", "0")))
    res = run_bass_kernel_spmd(nc, in_maps, list(range(B)), trace=trace)
    _CACHE["last_results"] = res
    return np.stack(
        [res.results[b]["out"].T.astype(np.float32) for b in range(B)],
        axis=0)


# revision 26
# speedup vs baseline: 1.2641x; 1.2641x over previous
"""Trainium2 Bass kernel for nn_MultiHeadAttention_36009005810143.

Data-parallel over batch B=8 across 8 NeuronCores; projection weights
replicated.  Per core: x [1024,640] -> MHA (10 heads, d=64, strict
causal additive -10000 mask) -> out [1024,640] * mask.

v3 design:
 - x transposed on the host; projections read x^T directly.
 - S^T = K_h^T Q_h causally trimmed (only q >= 128*kb per k-block),
   packed into bank-aligned slots of a [128, 2048] psum tile whose
   halves hold the two heads of a pair.  Head pairs live at partition
   offsets 0/64 of QT/KT, so the K=64-contraction S matmuls row-tile
   both heads concurrently on the PE array.
 - exp runs straight from PSUM in one ACTIVATE per 2048-col tile
   (scale=1/8); the causal band is zeroed AFTER exp on the gpsimd
   engine (exp(s-10000) == 0 exactly).
 - PV keeps the S^T orientation: stationary = [V_h | 1] (one cheap
   65-col LDWEIGHTS per (head, k-block)), moving = exp'd attention.
   Output lands transposed ([U, T]); the softmax denominator is row 64
   of each PV psum tile.  Den rows are DMA-gathered into a [2, T]
   staging tile, rden = mask / den in one DVE divide (folds the query
   mask: mask=0 -> 0), broadcast across partitions by the DMA engine,
   and applied in a single [64, T] multiply per head.  The host
   transposes the returned [U, T] array (layout only).
 - reference quirk: for q==0 every key gets -10000, so softmax over
   ALL 1024 keys of the raw scores.  S^T columns q=0..7 for k-blocks
   1..7 are 8-wide strips (cols 1..7 zeroed post-exp) accumulated into
   columns 0..7 of the qc=0 PV psum.
"""

import os
import sys
import types

import numpy as np

# The agent image's `antenv` package lacks `axon_hooks`, which
# concourse.bass_utils imports unconditionally when trace=True under
# axon.  Provide it (and register the real NTFF hook when available).
try:
    import antenv

    if not hasattr(antenv, "axon_hooks"):
        _hooks_mod = types.ModuleType("antenv.axon_hooks")
        _hooks_mod._hook = None

        def _set_hook(h):
            _hooks_mod._hook = h

        def _get_hook():
            return _hooks_mod._hook

        _hooks_mod.set_axon_ntff_profile_hook = _set_hook
        _hooks_mod.get_axon_ntff_profile_hook = _get_hook
        sys.modules["antenv.axon_hooks"] = _hooks_mod
        antenv.axon_hooks = _hooks_mod
        try:
            from trn_agent_boot.trn_boot import _ntff_profile_via_ctypes

            _set_hook(_ntff_profile_via_ctypes("/opt/axon/libaxon_pjrt.so"))
        except Exception:
            pass
except Exception:
    pass

import concourse.bass as bass
import concourse.mybir as mybir
import concourse.tile as tile
from concourse import bacc
from concourse.bass_utils import run_bass_kernel_spmd

F32 = mybir.dt.float32
F16 = mybir.dt.float16
AF = mybir.ActivationFunctionType

B, T, D, U, H, DH = 8, 1024, 640, 640, 10, 64
NTB = T // 128   # 8 t/q/k 128-blocks
NDB = D // 128   # 5 contraction blocks
NUB = U // 128   # 5 feature blocks
VCW = 320        # V projection chunk (5 heads)
PAIRS = H // 2   # 5 row-tiled head pairs
HD1 = DH + 1     # 65: head cols in Vg / PV psum partitions

# Per-head S^T slot layout: 5 logical tiles of [128, 1024] packed with
# causally-needed (kb, q-range) chunks; every matmul stays inside one
# 2KB psum bank.  Entries: (tile_idx, col_off, kb, q_lo, width)
CHUNKS = [
    (0, 0,   0, 0,   512), (0, 512, 0, 512, 512),
    (1, 0,   1, 128, 512), (1, 512, 1, 640, 384), (1, 896, 7, 896, 128),
    (2, 0,   2, 256, 512), (2, 512, 2, 768, 256), (2, 768, 6, 768, 256),
    (3, 0,   3, 384, 512), (3, 512, 3, 896, 128), (3, 640, 5, 640, 384),
    (4, 0,   4, 512, 512),
]
NS = 5           # S tiles per head
SW = 1024        # per-head S tile width (half of the [128,2048] psum)
STRIP0 = 512     # strip cols in tile 4: kb=1..7 at STRIP0 + 8*(kb-1)

# piece map: kb -> list of (tile, col_off, q_lo, q_hi)
_PIECES = {kb: [] for kb in range(NTB)}
for (ti, co, kb, qlo, w) in CHUNKS:
    _PIECES[kb].append((ti, co, qlo, qlo + w))

# diag regions: (tile, col_off) of the 128-wide diagonal block per kb
DIAG = {}
for (ti, co, kb, qlo, w) in CHUNKS:
    if qlo == 128 * kb:
        DIAG[kb] = (ti, co)


def _ptoff(hi, ti):
    """p_t column base for head-in-pair hi, S tile ti."""
    return 2048 * ti + 1024 * hi


def _pv_segments(qc):
    """PV moving-operand segments for one 512-col q chunk.
    Returns [(kb, ti, col_in_tile, psum_off, width)] with kb ascending;
    kb=0 comes first and spans the full 512 columns."""
    segs = []
    for kb in range(NTB):
        qlo, qhi = max(512 * qc, 128 * kb), 512 * (qc + 1)
        if qlo >= qhi:
            continue
        for (ti, co, plo, phi) in _PIECES[kb]:
            a, b = max(qlo, plo), min(qhi, phi)
            if a < b:
                segs.append((kb, ti, co + (a - plo), a - 512 * qc, b - a))
    return segs


PV_SEGS = {qc: _pv_segments(qc) for qc in range(2)}

_CACHE: dict = {}


def _build_module():
    nc = bacc.Bacc("TRN2", target_bir_lowering=False, debug=False, num_devices=B)

    xT_d = nc.dram_tensor("x", [D, T], F16, kind="ExternalInput").ap()
    m_d = nc.dram_tensor("mask", [1, T], F32, kind="ExternalInput").ap()
    wq_d = nc.dram_tensor("Wq", [D, U], F16, kind="ExternalInput").ap()
    wk_d = nc.dram_tensor("Wk", [D, U], F16, kind="ExternalInput").ap()
    wv_d = nc.dram_tensor("Wv", [D, U], F16, kind="ExternalInput").ap()
    out_d = nc.dram_tensor("out", [U, T], F16, kind="ExternalOutput").ap()

    ts = bass.ts

    with tile.TileContext(nc) as tc:
        from contextlib import ExitStack

        with ExitStack() as ctx:
            consts = ctx.enter_context(tc.tile_pool(name="consts", bufs=1))
            sb = ctx.enter_context(tc.tile_pool(name="sb", bufs=1))
            ptp = ctx.enter_context(tc.tile_pool(name="ptp", bufs=2))
            otp = ctx.enter_context(tc.tile_pool(name="otp", bufs=3))
            ofp = ctx.enter_context(tc.tile_pool(name="ofp", bufs=3))
            rbp = ctx.enter_context(tc.tile_pool(name="rbp", bufs=3))
            dnp = ctx.enter_context(tc.tile_pool(name="dnp", bufs=2))
            pp = ctx.enter_context(tc.tile_pool(name="pp", bufs=2, space="PSUM"))
            spp = ctx.enter_context(tc.tile_pool(name="spp", bufs=1, space="PSUM"))
            pvp = ctx.enter_context(tc.tile_pool(name="pvp", bufs=2, space="PSUM"))

            # ---------------- input DMAs (x^T, W first) ----------------
            xT = [sb.tile([128, T], F16, tag=f"xT{i}", name=f"xT{i}") for i in range(NDB)]
            Wq = [sb.tile([128, U], F16, tag=f"wq{i}", name=f"wq{i}") for i in range(NDB)]
            Wk = [sb.tile([128, U], F16, tag=f"wk{i}", name=f"wk{i}") for i in range(NDB)]
            Wv = [sb.tile([128, U], F16, tag=f"wv{i}", name=f"wv{i}") for i in range(NDB)]
            QT = [sb.tile([128, T], F16, tag=f"QT{i}", name=f"QT{i}") for i in range(NUB)]
            KT = [sb.tile([128, T], F16, tag=f"KT{i}", name=f"KT{i}") for i in range(NUB)]
            Vg = [sb.tile([128, H * HD1], F16, tag=f"Vg{i}", name=f"Vg{i}")
                  for i in range(NTB)]

            # interleave x^T/Wq tiles so the first Q projection chain can
            # stream while later tiles are still in flight
            for i in range(NDB):
                nc.sync.dma_start(xT[i][:], xT_d[ts(i, 128), :])
                nc.sync.dma_start(Wq[i][:], wq_d[ts(i, 128), :])
            for i in range(NDB):
                nc.sync.dma_start(Wk[i][:], wk_d[ts(i, 128), :])
            for i in range(NDB):
                nc.sync.dma_start(Wv[i][:], wv_d[ts(i, 128), :])

            mrow = consts.tile([1, T], F32, name="mrow")
            nc.sync.dma_start(mrow[:], m_d[:, :])
            # query mask broadcast across the PV psum partitions; row 64
            # stays 1.0 so the denominator is never masked (avoids 0/0).
            maskb = consts.tile([HD1, T], F32, name="maskb")
            nc.gpsimd.partition_broadcast(maskb[:], mrow[:], channels=HD1)
            nc.gpsimd.memset(maskb[DH:HD1, :], 1.0)

            ones_t = consts.tile([128, H], F32, name="ones_t")
            nc.vector.memset(ones_t[:], 1.0)
            # strict-causal triangular mask (1 where col > part, else 0)
            # for DVE-side band zeroing of head B
            tri = consts.tile([128, 128], F16, name="tri")
            nc.gpsimd.memset(tri[:], 1.0)
            nc.gpsimd.affine_select(
                out=tri[:], in_=tri[:],
                compare_op=mybir.AluOpType.is_ge,
                fill=0.0, base=-1,
                pattern=[[1, 128]], channel_multiplier=-1)
            for tb in range(NTB):
                oc = Vg[tb][:].rearrange("p (g c) -> p g c", c=HD1)[:, :, DH:HD1]
                nc.vector.tensor_copy(
                    oc, ones_t[:].rearrange("p (g c) -> p g c", c=1))

            # ---------------- projection helpers -----------------------
            def proj_qk(W, dst, ub, qc, drain_act=False):
                ps = pp.tile([128, 512], F32, tag="pp", name="pp")
                for db in range(NDB):
                    nc.tensor.matmul(
                        ps[:], W[db][:, ts(ub, 128)], xT[db][:, ts(qc, 512)],
                        start=(db == 0), stop=(db == NDB - 1))
                if drain_act:
                    nc.scalar.activation(dst[ub][:, ts(qc, 512)], ps[:], AF.Copy)
                else:
                    nc.vector.tensor_copy(dst[ub][:, ts(qc, 512)], ps[:])

            def proj_v(tb, vc, drain_act=False):
                ps = pp.tile([128, 512], F32, tag="pp", name="pp")
                for db in range(NDB):
                    nc.tensor.matmul(
                        ps[:, 0:VCW], xT[db][:, ts(tb, 128)],
                        Wv[db][:, ts(vc, VCW)],
                        start=(db == 0), stop=(db == NDB - 1))
                dst = Vg[tb][:, vc * 5 * HD1:(vc + 1) * 5 * HD1]
                dst = dst.rearrange("p (g c) -> p g c", c=HD1)[:, :, 0:DH]
                src = ps[:, 0:VCW].rearrange("p (g c) -> p g c", c=DH)
                if drain_act:
                    nc.scalar.activation(dst, src, AF.Copy)
                else:
                    nc.vector.tensor_copy(dst, src)

            # ---------------- attention emission -----------------------
            PB = {}    # pair -> ptAB tile

            def emit_s_tile(pair, ti):
                kt = KT[pair]
                qt = QT[pair]
                sAB = spp.tile([128, 2048], F32, tag="sAB", name="sAB")
                for (t2, co, kb, qlo, w) in CHUNKS:
                    if t2 != ti:
                        continue
                    for hi in range(2):
                        po = DH * hi
                        nc.tensor.matmul(
                            sAB[:, 1024 * hi + co:1024 * hi + co + w],
                            kt[po:po + DH, ts(kb, 128)],
                            qt[po:po + DH, qlo:qlo + w],
                            start=True, stop=True)
                if ti == 4:
                    for kb in range(1, NTB):
                        co = STRIP0 + 8 * (kb - 1)
                        for hi in range(2):
                            po = DH * hi
                            nc.tensor.matmul(
                                sAB[:, 1024 * hi + co:1024 * hi + co + 8],
                                kt[po:po + DH, ts(kb, 128)],
                                qt[po:po + DH, 0:8],
                                start=True, stop=True)
                return sAB

            def emit_exp_tile(pair, ti, sAB):
                pt = PB[pair]
                # one exp call per head-half: the next S tile's A-half
                # matmuls only wait on the A-half exp, hiding the refill
                # under the B-half exp
                w = 1024 if ti < 4 else STRIP0 + 56
                for hi in range(2):
                    nc.scalar.activation(
                        pt[:, _ptoff(hi, ti):_ptoff(hi, ti) + w],
                        sAB[:, 1024 * hi:1024 * hi + w],
                        AF.Exp, scale=0.125)
                # post-exp band zeroing: head A on gpsimd, head B on DVE
                # (triangular-mask multiply) to split the load
                for hi in range(2):
                    for kb in range(NTB):
                        t2, co = DIAG[kb]
                        if t2 != ti:
                            continue
                        base = _ptoff(hi, ti) + co
                        if kb == 0:
                            # keep col q=0 (reference quirk); cols 1..127:
                            # keep iff (c+1) > p  <=>  c - p >= 0
                            if hi == 0:
                                nc.gpsimd.affine_select(
                                    out=pt[:, base + 1:base + 128],
                                    in_=pt[:, base + 1:base + 128],
                                    compare_op=mybir.AluOpType.is_ge,
                                    fill=0.0, base=0,
                                    pattern=[[1, 127]], channel_multiplier=-1)
                            else:
                                nc.vector.tensor_tensor(
                                    pt[:, base + 1:base + 128],
                                    pt[:, base + 1:base + 128],
                                    tri[:, 1:128],
                                    op=mybir.AluOpType.mult)
                        else:
                            # keep iff c - p - 1 >= 0 (strict causal)
                            if hi == 0:
                                nc.gpsimd.affine_select(
                                    out=pt[:, base:base + 128],
                                    in_=pt[:, base:base + 128],
                                    compare_op=mybir.AluOpType.is_ge,
                                    fill=0.0, base=-1,
                                    pattern=[[1, 128]], channel_multiplier=-1)
                            else:
                                nc.vector.tensor_tensor(
                                    pt[:, base:base + 128],
                                    pt[:, base:base + 128],
                                    tri[:],
                                    op=mybir.AluOpType.mult)
                    if ti == 4:
                        sb0 = _ptoff(hi, 4) + STRIP0
                        # zero cols 1..7 of each 8-wide q0 strip
                        nc.gpsimd.affine_select(
                            out=pt[:, sb0:sb0 + 56].rearrange(
                                "p (g c) -> p g c", c=8),
                            in_=pt[:, sb0:sb0 + 56].rearrange(
                                "p (g c) -> p g c", c=8),
                            compare_op=mybir.AluOpType.is_ge,
                            fill=0.0, base=0,
                            pattern=[[0, 7], [-1, 8]], channel_multiplier=0)

            def emit_pv_chain(pair, hi, qc, oT, sden):
                """PV chain for head 2*pair+hi over q chunk qc; drains into
                oT with the query mask folded in, then DMA-gathers the den
                row into the head's base-0 staging tile."""
                pt = PB[pair]
                h = 2 * pair + hi
                pv = pvp.tile([HD1, 512], F32, tag="pv", name="pv")
                vsl = (h * HD1, (h + 1) * HD1)
                segs = PV_SEGS[qc]
                for j, (kb, ti, c, off, w) in enumerate(segs):
                    nc.tensor.matmul(
                        pv[:, off:off + w],
                        Vg[kb][:, vsl[0]:vsl[1]],
                        pt[:, _ptoff(hi, ti) + c:_ptoff(hi, ti) + c + w],
                        start=(j == 0),
                        stop=(qc == 1 and j == len(segs) - 1))
                if qc == 0:
                    for kb in range(1, NTB):
                        c = _ptoff(hi, 4) + STRIP0 + 8 * (kb - 1)
                        nc.tensor.matmul(
                            pv[:, 0:8],
                            Vg[kb][:, vsl[0]:vsl[1]],
                            pt[:, c:c + 8],
                            start=False, stop=(kb == NTB - 1))
                nc.vector.tensor_tensor(
                    oT[:, ts(qc, 512)], pv[:], maskb[:, ts(qc, 512)],
                    op=mybir.AluOpType.mult)
                nc.sync.dma_start(
                    sden[:, ts(qc, 512)], oT[64:65, ts(qc, 512)])

            HS = {}  # (pair, hi) -> (oT, sden) from the chain unit

            def emit_head_chains(pair, hi):
                oT = otp.tile([HD1, T], F32, tag="oT", name="oT")
                sden = dnp.tile([1, T], F32, tag="sden", name="sden")
                for qc in range(2):
                    emit_pv_chain(pair, hi, qc, oT, sden)
                HS[(pair, hi)] = (oT, sden)

            def emit_head_norm(pair, hi):
                oT, sden = HS.pop((pair, hi))
                dbc = rbp.tile([DH, T], F32, tag="dbc", name="dbc")
                nc.gpsimd.partition_broadcast(
                    dbc[:], sden[:], channels=DH)
                # den is a sum of exps: positive, finite -> approx ok
                nc.vector.reciprocal_approx_fast(dbc[:], dbc[:])
                oF = ofp.tile([DH, T], F16, tag="oF", name="oF")
                nc.vector.tensor_tensor(
                    oF[:], oT[0:DH, :], dbc[:],
                    op=mybir.AluOpType.mult)
                h = 2 * pair + hi
                nc.sync.dma_start(out_d[h * DH:(h + 1) * DH, :], oF[:])

            # ------------------- emission schedule ---------------------
            proj_qk(Wq, QT, 0, 0, True); proj_qk(Wq, QT, 0, 1, True)
            proj_qk(Wk, KT, 0, 0, True); proj_qk(Wk, KT, 0, 1, True)
            for tb in range(NTB):
                proj_v(tb, 0, drain_act=True)

            # deferred work: (est_pe_us, needed_by_pair, emit_fn)
            work = []
            for ub in range(1, NUB):
                for (W, DST) in ((Wq, QT), (Wk, KT)):
                    for qc in range(2):
                        work.append((1.1, ub, lambda W=W, D=DST, u=ub, q=qc:
                                     proj_qk(W, D, u, q)))
            for tb in range(NTB):
                work.append((0.7, None, lambda t=tb: proj_v(t, 1)))

            def emit_work(budget_us):
                spent = 0.0
                while work and spent < budget_us:
                    cost, _, fn = work.pop(0)
                    fn()
                    spent += cost

            def flush_proj_for(pair):
                keep = []
                for (cost, ub, fn) in work:
                    if ub is not None and ub <= pair:
                        fn()
                    else:
                        keep.append((cost, ub, fn))
                work[:] = keep

            for pair in range(PAIRS):
                PB[pair] = ptp.tile([128, 2048 * NS], F16, tag="ptAB", name="ptAB")
                flush_proj_for(pair)
                for ti in range(NS):
                    sAB = emit_s_tile(pair, ti)
                    emit_exp_tile(pair, ti, sAB)
                    emit_work(2.2)
                # pair's own head A becomes available right after its exps;
                # queue it first so the next gaps pick it up immediately
                work.insert(0, (2.0, None,
                                lambda p=pair: emit_head_chains(p, 0)))
                work.insert(1, (0.3, None,
                                lambda p=pair: emit_head_norm(p, 0)))
                work.append((2.0, None,
                             lambda p=pair: emit_head_chains(p, 1)))
                work.append((0.3, None,
                             lambda p=pair: emit_head_norm(p, 1)))
                emit_work(2.3)

            while work:
                _, _, fn = work.pop(0)
                fn()

    nc.compile()
    return nc


def get_nc():
    if "nc" not in _CACHE:
        _CACHE["nc"] = _build_module()
    return _CACHE["nc"]


def kernel(x, mask, Wq, Wk, Wv):
    x = np.asarray(x, dtype=np.float32).astype(np.float16)
    xT = np.ascontiguousarray(x.transpose(0, 2, 1))  # [B, D, T]
    mask_f = np.ascontiguousarray(
        np.asarray(mask).astype(np.float32).reshape(B, 1, T))
    Wq = np.ascontiguousarray(np.asarray(Wq, dtype=np.float32).astype(np.float16))
    Wk = np.ascontiguousarray(np.asarray(Wk, dtype=np.float32).astype(np.float16))
    Wv = np.ascontiguousarray(np.asarray(Wv, dtype=np.float32).astype(np.float16))

    nc = get_nc()
    in_maps = [
        {"x": xT[b], "mask": mask_f[b], "Wq": Wq, "Wk": Wk, "Wv": Wv}
        for b in range(B)
    ]
    trace = bool(int(os.environ.get("KERNEL_TRACE", "0")))
    res = run_bass_kernel_spmd(nc, in_maps, list(range(B)), trace=trace)
    _CACHE["last_results"] = res
    return np.stack(
        [res.results[b]["out"].T.astype(np.float32) for b in range(B)],
        axis=0)


# revision 29
# speedup vs baseline: 1.3544x; 1.0715x over previous
"""Trainium2 Bass kernel for nn_MultiHeadAttention_36009005810143.

Data-parallel over batch B=8 across 8 NeuronCores; projection weights
replicated.  Per core: x [1024,640] -> MHA (10 heads, d=64, strict
causal additive -10000 mask) -> out [1024,640] * mask.

v3 design:
 - x transposed on the host; projections read x^T directly.
 - S^T = K_h^T Q_h causally trimmed (only q >= 128*kb per k-block),
   packed into bank-aligned slots of a [128, 2048] psum tile whose
   halves hold the two heads of a pair.  Head pairs live at partition
   offsets 0/64 of QT/KT, so the K=64-contraction S matmuls row-tile
   both heads concurrently on the PE array.
 - exp runs straight from PSUM in one ACTIVATE per 2048-col tile
   (scale=1/8); the causal band is zeroed AFTER exp on the gpsimd
   engine (exp(s-10000) == 0 exactly).
 - PV keeps the S^T orientation: stationary = [V_h | 1] (one cheap
   65-col LDWEIGHTS per (head, k-block)), moving = exp'd attention.
   Output lands transposed ([U, T]); the softmax denominator is row 64
   of each PV psum tile.  Den rows are DMA-gathered into a [2, T]
   staging tile, rden = mask / den in one DVE divide (folds the query
   mask: mask=0 -> 0), broadcast across partitions by the DMA engine,
   and applied in a single [64, T] multiply per head.  The host
   transposes the returned [U, T] array (layout only).
 - reference quirk: for q==0 every key gets -10000, so softmax over
   ALL 1024 keys of the raw scores.  S^T columns q=0..7 for k-blocks
   1..7 are 8-wide strips (cols 1..7 zeroed post-exp) accumulated into
   columns 0..7 of the qc=0 PV psum.
"""

import os
import sys
import types

import numpy as np

# The agent image's `antenv` package lacks `axon_hooks`, which
# concourse.bass_utils imports unconditionally when trace=True under
# axon.  Provide it (and register the real NTFF hook when available).
try:
    import antenv

    if not hasattr(antenv, "axon_hooks"):
        _hooks_mod = types.ModuleType("antenv.axon_hooks")
        _hooks_mod._hook = None

        def _set_hook(h):
            _hooks_mod._hook = h

        def _get_hook():
            return _hooks_mod._hook

        _hooks_mod.set_axon_ntff_profile_hook = _set_hook
        _hooks_mod.get_axon_ntff_profile_hook = _get_hook
        sys.modules["antenv.axon_hooks"] = _hooks_mod
        antenv.axon_hooks = _hooks_mod
        try:
            from trn_agent_boot.trn_boot import _ntff_profile_via_ctypes

            _set_hook(_ntff_profile_via_ctypes("/opt/axon/libaxon_pjrt.so"))
        except Exception:
            pass
except Exception:
    pass

import concourse.bass as bass
import concourse.mybir as mybir
import concourse.tile as tile
from concourse import bacc
from concourse.bass_utils import run_bass_kernel_spmd

F32 = mybir.dt.float32
F16 = mybir.dt.float16
AF = mybir.ActivationFunctionType

B, T, D, U, H, DH = 8, 1024, 640, 640, 10, 64
NTB = T // 128   # 8 t/q/k 128-blocks
NDB = D // 128   # 5 contraction blocks
NUB = U // 128   # 5 feature blocks
VCW = 320        # V projection chunk (5 heads)
PAIRS = H // 2   # 5 row-tiled head pairs
HD1 = DH + 1     # 65: head cols in Vg / PV psum partitions

# Per-head S^T slot layout: 5 logical tiles of [128, 1024] packed with
# causally-needed (kb, q-range) chunks; every matmul stays inside one
# 2KB psum bank.  Entries: (tile_idx, col_off, kb, q_lo, width)
CHUNKS = [
    (0, 0,   0, 0,   512), (0, 512, 0, 512, 512),
    (1, 0,   1, 128, 512), (1, 512, 1, 640, 384), (1, 896, 7, 896, 128),
    (2, 0,   2, 256, 512), (2, 512, 2, 768, 256), (2, 768, 6, 768, 256),
    (3, 0,   3, 384, 512), (3, 512, 3, 896, 128), (3, 640, 5, 640, 384),
    (4, 0,   4, 512, 512),
]
NS = 5           # S tiles per head
SW = 1024        # per-head S tile width (half of the [128,2048] psum)
STRIP0 = 512     # strip cols in tile 4: kb=1..7 at STRIP0 + 8*(kb-1)

# piece map: kb -> list of (tile, col_off, q_lo, q_hi)
_PIECES = {kb: [] for kb in range(NTB)}
for (ti, co, kb, qlo, w) in CHUNKS:
    _PIECES[kb].append((ti, co, qlo, qlo + w))

# diag regions: (tile, col_off) of the 128-wide diagonal block per kb
DIAG = {}
for (ti, co, kb, qlo, w) in CHUNKS:
    if qlo == 128 * kb:
        DIAG[kb] = (ti, co)


def _ptoff(hi, ti):
    """p_t column base for head-in-pair hi, S tile ti."""
    return 2048 * ti + 1024 * hi


def _pv_segments(qc):
    """PV moving-operand segments for one 512-col q chunk.
    Returns [(kb, ti, col_in_tile, psum_off, width)] with kb ascending;
    kb=0 comes first and spans the full 512 columns."""
    segs = []
    for kb in range(NTB):
        qlo, qhi = max(512 * qc, 128 * kb), 512 * (qc + 1)
        if qlo >= qhi:
            continue
        for (ti, co, plo, phi) in _PIECES[kb]:
            a, b = max(qlo, plo), min(qhi, phi)
            if a < b:
                segs.append((kb, ti, co + (a - plo), a - 512 * qc, b - a))
    return segs


PV_SEGS = {qc: _pv_segments(qc) for qc in range(2)}

_CACHE: dict = {}


def _build_module():
    nc = bacc.Bacc("TRN2", target_bir_lowering=False, debug=False, num_devices=B)

    xT_d = nc.dram_tensor("x", [D, T], F16, kind="ExternalInput").ap()
    m_d = nc.dram_tensor("mask", [1, T], F32, kind="ExternalInput").ap()
    wq_d = nc.dram_tensor("Wq", [D, U], F16, kind="ExternalInput").ap()
    wk_d = nc.dram_tensor("Wk", [D, U], F16, kind="ExternalInput").ap()
    wv_d = nc.dram_tensor("Wv", [D, U], F16, kind="ExternalInput").ap()
    out_d = nc.dram_tensor("out", [U, T], F16, kind="ExternalOutput").ap()

    ts = bass.ts

    with tile.TileContext(nc) as tc:
        from contextlib import ExitStack

        with ExitStack() as ctx:
            consts = ctx.enter_context(tc.tile_pool(name="consts", bufs=1))
            sb = ctx.enter_context(tc.tile_pool(name="sb", bufs=1))
            ptp = ctx.enter_context(tc.tile_pool(name="ptp", bufs=2))
            otp = ctx.enter_context(tc.tile_pool(name="otp", bufs=3))
            ofp = ctx.enter_context(tc.tile_pool(name="ofp", bufs=3))
            rbp = ctx.enter_context(tc.tile_pool(name="rbp", bufs=3))
            dnp = ctx.enter_context(tc.tile_pool(name="dnp", bufs=2))
            pp = ctx.enter_context(tc.tile_pool(name="pp", bufs=2, space="PSUM"))
            spA = ctx.enter_context(tc.tile_pool(name="spA", bufs=1, space="PSUM"))
            spB = ctx.enter_context(tc.tile_pool(name="spB", bufs=1, space="PSUM"))
            pvp = ctx.enter_context(tc.tile_pool(name="pvp", bufs=2, space="PSUM"))

            # ---------------- input DMAs (x^T, W first) ----------------
            xT = [sb.tile([128, T], F16, tag=f"xT{i}", name=f"xT{i}") for i in range(NDB)]
            Wq = [sb.tile([128, U], F16, tag=f"wq{i}", name=f"wq{i}") for i in range(NDB)]
            Wk = [sb.tile([128, U], F16, tag=f"wk{i}", name=f"wk{i}") for i in range(NDB)]
            Wv = [sb.tile([128, U], F16, tag=f"wv{i}", name=f"wv{i}") for i in range(NDB)]
            QT = [sb.tile([128, T], F16, tag=f"QT{i}", name=f"QT{i}") for i in range(NUB)]
            KT = [sb.tile([128, T], F16, tag=f"KT{i}", name=f"KT{i}") for i in range(NUB)]
            Vg = [sb.tile([128, H * HD1], F16, tag=f"Vg{i}", name=f"Vg{i}")
                  for i in range(NTB)]

            # interleave x^T/Wq tiles so the first Q projection chain can
            # stream while later tiles are still in flight
            for i in range(NDB):
                nc.sync.dma_start(xT[i][:], xT_d[ts(i, 128), :])
                nc.sync.dma_start(Wq[i][:], wq_d[ts(i, 128), :])
            for i in range(NDB):
                nc.sync.dma_start(Wk[i][:], wk_d[ts(i, 128), :])
            for i in range(NDB):
                nc.sync.dma_start(Wv[i][:], wv_d[ts(i, 128), :])

            mrow = consts.tile([1, T], F32, name="mrow")
            nc.sync.dma_start(mrow[:], m_d[:, :])
            # query mask broadcast across the PV psum partitions; row 64
            # stays 1.0 so the denominator is never masked (avoids 0/0).
            maskb = consts.tile([HD1, T], F32, name="maskb")
            nc.gpsimd.partition_broadcast(maskb[:], mrow[:], channels=HD1)
            nc.gpsimd.memset(maskb[DH:HD1, :], 1.0)

            ones_t = consts.tile([128, H], F32, name="ones_t")
            nc.vector.memset(ones_t[:], 1.0)
            # strict-causal triangular mask (1 where col > part, else 0)
            # for DVE-side band zeroing of head B
            tri = consts.tile([128, 128], F16, name="tri")
            nc.gpsimd.memset(tri[:], 1.0)
            nc.gpsimd.affine_select(
                out=tri[:], in_=tri[:],
                compare_op=mybir.AluOpType.is_ge,
                fill=0.0, base=-1,
                pattern=[[1, 128]], channel_multiplier=-1)
            for tb in range(NTB):
                oc = Vg[tb][:].rearrange("p (g c) -> p g c", c=HD1)[:, :, DH:HD1]
                nc.vector.tensor_copy(
                    oc, ones_t[:].rearrange("p (g c) -> p g c", c=1))

            # ---------------- projection helpers -----------------------
            def proj_qk(W, dst, ub, qc, drain_act=False):
                ps = pp.tile([128, 512], F32, tag="pp", name="pp")
                for db in range(NDB):
                    nc.tensor.matmul(
                        ps[:], W[db][:, ts(ub, 128)], xT[db][:, ts(qc, 512)],
                        start=(db == 0), stop=(db == NDB - 1))
                if drain_act:
                    nc.scalar.activation(dst[ub][:, ts(qc, 512)], ps[:], AF.Copy)
                else:
                    nc.vector.tensor_copy(dst[ub][:, ts(qc, 512)], ps[:])

            def proj_v(tb, vc, drain_act=False):
                ps = pp.tile([128, 512], F32, tag="pp", name="pp")
                for db in range(NDB):
                    nc.tensor.matmul(
                        ps[:, 0:VCW], xT[db][:, ts(tb, 128)],
                        Wv[db][:, ts(vc, VCW)],
                        start=(db == 0), stop=(db == NDB - 1))
                dst = Vg[tb][:, vc * 5 * HD1:(vc + 1) * 5 * HD1]
                dst = dst.rearrange("p (g c) -> p g c", c=HD1)[:, :, 0:DH]
                src = ps[:, 0:VCW].rearrange("p (g c) -> p g c", c=DH)
                if drain_act:
                    nc.scalar.activation(dst, src, AF.Copy)
                else:
                    nc.vector.tensor_copy(dst, src)

            # ---------------- attention emission -----------------------
            PB = {}    # pair -> ptAB tile

            def emit_s_tile(pair, ti):
                kt = KT[pair]
                qt = QT[pair]
                # separate per-head psum tiles: the head-A tile frees as
                # soon as its own exp completes, so the next tile's A fill
                # hides under the head-B exp (ring-2 pipelining)
                sA = spA.tile([128, 1024], F32, tag="sA", name="sA")
                sB = spB.tile([128, 1024], F32, tag="sB", name="sB")
                sH = (sA, sB)
                for (t2, co, kb, qlo, w) in CHUNKS:
                    if t2 != ti:
                        continue
                    for hi in range(2):
                        po = DH * hi
                        nc.tensor.matmul(
                            sH[hi][:, co:co + w],
                            kt[po:po + DH, ts(kb, 128)],
                            qt[po:po + DH, qlo:qlo + w],
                            start=True, stop=True)
                if ti == 4:
                    for kb in range(1, NTB):
                        co = STRIP0 + 8 * (kb - 1)
                        for hi in range(2):
                            po = DH * hi
                            nc.tensor.matmul(
                                sH[hi][:, co:co + 8],
                                kt[po:po + DH, ts(kb, 128)],
                                qt[po:po + DH, 0:8],
                                start=True, stop=True)
                return sH

            def emit_exp_tile(pair, ti, sH):
                pt = PB[pair]
                w = 1024 if ti < 4 else STRIP0 + 56
                for hi in range(2):
                    nc.scalar.activation(
                        pt[:, _ptoff(hi, ti):_ptoff(hi, ti) + w],
                        sH[hi][:, 0:w],
                        AF.Exp, scale=0.125)
                # post-exp band zeroing: head A on gpsimd, head B on DVE
                # (triangular-mask multiply) to split the load
                for hi in range(2):
                    for kb in range(NTB):
                        t2, co = DIAG[kb]
                        if t2 != ti:
                            continue
                        base = _ptoff(hi, ti) + co
                        if kb == 0:
                            # keep col q=0 (reference quirk); cols 1..127:
                            # keep iff (c+1) > p  <=>  c - p >= 0
                            if hi == 0:
                                nc.gpsimd.affine_select(
                                    out=pt[:, base + 1:base + 128],
                                    in_=pt[:, base + 1:base + 128],
                                    compare_op=mybir.AluOpType.is_ge,
                                    fill=0.0, base=0,
                                    pattern=[[1, 127]], channel_multiplier=-1)
                            else:
                                nc.vector.tensor_tensor(
                                    pt[:, base + 1:base + 128],
                                    pt[:, base + 1:base + 128],
                                    tri[:, 1:128],
                                    op=mybir.AluOpType.mult)
                        else:
                            # keep iff c - p - 1 >= 0 (strict causal)
                            if hi == 0:
                                nc.gpsimd.affine_select(
                                    out=pt[:, base:base + 128],
                                    in_=pt[:, base:base + 128],
                                    compare_op=mybir.AluOpType.is_ge,
                                    fill=0.0, base=-1,
                                    pattern=[[1, 128]], channel_multiplier=-1)
                            else:
                                nc.vector.tensor_tensor(
                                    pt[:, base:base + 128],
                                    pt[:, base:base + 128],
                                    tri[:],
                                    op=mybir.AluOpType.mult)
                    if ti == 4:
                        sb0 = _ptoff(hi, 4) + STRIP0
                        # zero cols 1..7 of each 8-wide q0 strip
                        nc.gpsimd.affine_select(
                            out=pt[:, sb0:sb0 + 56].rearrange(
                                "p (g c) -> p g c", c=8),
                            in_=pt[:, sb0:sb0 + 56].rearrange(
                                "p (g c) -> p g c", c=8),
                            compare_op=mybir.AluOpType.is_ge,
                            fill=0.0, base=0,
                            pattern=[[0, 7], [-1, 8]], channel_multiplier=0)

            def emit_pv_chain(pair, hi, qc, oT, sden):
                """PV chain for head 2*pair+hi over q chunk qc; drains into
                oT with the query mask folded in, then DMA-gathers the den
                row into the head's base-0 staging tile."""
                pt = PB[pair]
                h = 2 * pair + hi
                pv = pvp.tile([HD1, 512], F32, tag="pv", name="pv")
                vsl = (h * HD1, (h + 1) * HD1)
                segs = PV_SEGS[qc]
                for j, (kb, ti, c, off, w) in enumerate(segs):
                    nc.tensor.matmul(
                        pv[:, off:off + w],
                        Vg[kb][:, vsl[0]:vsl[1]],
                        pt[:, _ptoff(hi, ti) + c:_ptoff(hi, ti) + c + w],
                        start=(j == 0),
                        stop=(qc == 1 and j == len(segs) - 1))
                if qc == 0:
                    for kb in range(1, NTB):
                        c = _ptoff(hi, 4) + STRIP0 + 8 * (kb - 1)
                        nc.tensor.matmul(
                            pv[:, 0:8],
                            Vg[kb][:, vsl[0]:vsl[1]],
                            pt[:, c:c + 8],
                            start=False, stop=(kb == NTB - 1))
                nc.vector.tensor_tensor(
                    oT[:, ts(qc, 512)], pv[:], maskb[:, ts(qc, 512)],
                    op=mybir.AluOpType.mult)
                nc.sync.dma_start(
                    sden[:, ts(qc, 512)], oT[64:65, ts(qc, 512)])

            HS = {}  # (pair, hi) -> (oT, sden) from the chain unit

            def emit_head_chains(pair, hi):
                oT = otp.tile([HD1, T], F32, tag="oT", name="oT")
                sden = dnp.tile([1, T], F32, tag="sden", name="sden")
                for qc in range(2):
                    emit_pv_chain(pair, hi, qc, oT, sden)
                HS[(pair, hi)] = (oT, sden)

            def emit_head_norm(pair, hi):
                oT, sden = HS.pop((pair, hi))
                dbc = rbp.tile([DH, T], F32, tag="dbc", name="dbc")
                nc.gpsimd.partition_broadcast(
                    dbc[:], sden[:], channels=DH)
                # den is a sum of exps: positive, finite -> approx ok
                nc.vector.reciprocal_approx_fast(dbc[:], dbc[:])
                oF = ofp.tile([DH, T], F16, tag="oF", name="oF")
                nc.vector.tensor_tensor(
                    oF[:], oT[0:DH, :], dbc[:],
                    op=mybir.AluOpType.mult)
                h = 2 * pair + hi
                nc.sync.dma_start(out_d[h * DH:(h + 1) * DH, :], oF[:])

            # ------------------- emission schedule ---------------------
            proj_qk(Wq, QT, 0, 0, True); proj_qk(Wq, QT, 0, 1, True)
            proj_qk(Wk, KT, 0, 0, True); proj_qk(Wk, KT, 0, 1, True)
            for tb in range(NTB):
                proj_v(tb, 0, drain_act=True)

            # deferred work: (est_pe_us, needed_by_pair, emit_fn)
            work = []
            for ub in range(1, NUB):
                for (W, DST) in ((Wq, QT), (Wk, KT)):
                    for qc in range(2):
                        work.append((1.1, ub, lambda W=W, D=DST, u=ub, q=qc:
                                     proj_qk(W, D, u, q)))
            for tb in range(NTB):
                work.append((0.7, None, lambda t=tb: proj_v(t, 1)))

            def emit_work(budget_us):
                spent = 0.0
                while work and spent < budget_us:
                    cost, _, fn = work.pop(0)
                    fn()
                    spent += cost

            def flush_proj_for(pair):
                keep = []
                for (cost, ub, fn) in work:
                    if ub is not None and ub <= pair:
                        fn()
                    else:
                        keep.append((cost, ub, fn))
                work[:] = keep

            for pair in range(PAIRS):
                PB[pair] = ptp.tile([128, 2048 * NS], F16, tag="ptAB", name="ptAB")
                flush_proj_for(pair)
                for ti in range(NS):
                    sH = emit_s_tile(pair, ti)
                    emit_exp_tile(pair, ti, sH)
                    emit_work(2.2)
                # pair's own head A becomes available right after its exps;
                # queue it first so the next gaps pick it up immediately
                work.insert(0, (2.0, None,
                                lambda p=pair: emit_head_chains(p, 0)))
                work.insert(1, (0.3, None,
                                lambda p=pair: emit_head_norm(p, 0)))
                work.append((2.0, None,
                             lambda p=pair: emit_head_chains(p, 1)))
                work.append((0.3, None,
                             lambda p=pair: emit_head_norm(p, 1)))
                emit_work(2.3)

            while work:
                _, _, fn = work.pop(0)
                fn()

    nc.compile()
    return nc


def get_nc():
    if "nc" not in _CACHE:
        _CACHE["nc"] = _build_module()
    return _CACHE["nc"]


def kernel(x, mask, Wq, Wk, Wv):
    x = np.asarray(x, dtype=np.float32).astype(np.float16)
    xT = np.ascontiguousarray(x.transpose(0, 2, 1))  # [B, D, T]
    mask_f = np.ascontiguousarray(
        np.asarray(mask).astype(np.float32).reshape(B, 1, T))
    Wq = np.ascontiguousarray(np.asarray(Wq, dtype=np.float32).astype(np.float16))
    Wk = np.ascontiguousarray(np.asarray(Wk, dtype=np.float32).astype(np.float16))
    Wv = np.ascontiguousarray(np.asarray(Wv, dtype=np.float32).astype(np.float16))

    nc = get_nc()
    in_maps = [
        {"x": xT[b], "mask": mask_f[b], "Wq": Wq, "Wk": Wk, "Wv": Wv}
        for b in range(B)
    ]
    trace = bool(int(os.environ.get("KERNEL_TRACE", "0")))
    res = run_bass_kernel_spmd(nc, in_maps, list(range(B)), trace=trace)
    _CACHE["last_results"] = res
    return np.stack(
        [res.results[b]["out"].T.astype(np.float32) for b in range(B)],
        axis=0)
